# revision 2
# baseline (speedup 1.0000x reference)
"""Sparse window attention (NMS-selected windows) for Trainium2, 8 cores.

Strategy:
- Host: replicate the (tiny) score/NMS control flow bit-exactly with jax-CPU,
  build per-batch gather/scatter token tables and 1/count patches.
- Device (1 batch per NeuronCore, SPMD over 8 cores): per selected window,
  indirect-DMA gather the 16x16 patch (token-major), ROI-align as a dense
  kron(BILIN,BILIN) matmul, qkv projection, attention (scores^T layout,
  softmax normalizer via an appended ones-column in V), output projection,
  multiply by 1/count and indirect-DMA scatter-accumulate into out (which is
  pre-initialized with x).
"""

import os
import sys

sys.path.insert(0, "/opt/trn_rl_repo")

_ABLATE = os.environ.get("KERNEL_ABLATE", "")

import numpy as np

import concourse.bass as bass
import concourse.bacc as bacc
import concourse.mybir as mybir
import concourse.tile as tile
from concourse.bass_utils import run_bass_kernel_spmd
from concourse.masks import make_identity

f32 = mybir.dt.float32
i32 = mybir.dt.int32

# ---- problem constants (must match reference.py) ----
H = W = 128
WIN = 16
DIM = 512
HEADS = 8
DIM_HEAD = 64
INNER = HEADS * DIM_HEAD
SCALE = DIM_HEAD ** -0.5
KEEP = 44
IOU_T = 0.2
NB = 8
NTOK = H * W  # 16384
M = 225

# static shifted windows
_s = np.arange(0, H - WIN + 1, WIN // 2)
_sx, _sy = np.meshgrid(_s, _s)
WINDOWS = np.stack(
    [_sx.ravel(), _sy.ravel(), _sx.ravel() + WIN - 1, _sy.ravel() + WIN - 1], 1
).astype(np.float32)
SX_NP = WINDOWS[:, 0].astype(np.int32)
SY_NP = WINDOWS[:, 1].astype(np.int32)

_x1, _y1, _x2, _y2 = WINDOWS[:, 0], WINDOWS[:, 1], WINDOWS[:, 2], WINDOWS[:, 3]
_area = (_x2 - _x1) * (_y2 - _y1)
_iw = np.clip(np.minimum(_x2[:, None], _x2[None]) - np.maximum(_x1[:, None], _x1[None]), 0, None)
_ih = np.clip(np.minimum(_y2[:, None], _y2[None]) - np.maximum(_y1[:, None], _y1[None]), 0, None)
_inter = _iw * _ih
IOU_NP = (_inter / (_area[:, None] + _area[None] - _inter)).astype(np.float32)

_bin = (WIN - 1.0) / WIN
_r = (np.arange(WIN) + 0.5) * _bin
_q = np.floor(_r).astype(int)
_f = (_r - _q).astype(np.float32)
_A = np.zeros((WIN, WIN), np.float32)
_A[np.arange(WIN), _q] = 1.0 - _f
_A[np.arange(WIN), np.minimum(_q + 1, WIN - 1)] += _f
BILIN_NP = _A  # (16,16)


def _nms_select_numpy(prob, w_fix):
    """Numpy fallback replica of the reference score+NMS."""
    entropy = -np.sum(prob * np.log2(prob + np.float32(1e-10)), axis=1)
    k = w_fix[0, 0]
    sc = np.zeros((NB, 15, 15), np.float32)
    for i in range(15):
        for j in range(15):
            sc[:, i, j] = np.sum(
                entropy[:, i * 4:i * 4 + 8, j * 4:j * 4 + 8] * k[None], axis=(1, 2)
            )
    score = (sc / np.float32(64.0)).reshape(NB, -1)
    out = np.zeros((NB, KEEP), np.int64)
    for b in range(NB):
        order = np.argsort(-score[b], kind="stable")
        iou_s = IOU_NP[order][:, order]
        supp = np.zeros(M, bool)
        for i in range(M):
            if not supp[i]:
                supp |= (iou_s[i] > IOU_T) & (np.arange(M) > i)
        pos = np.where(~supp, np.arange(M), M)
        pos = np.minimum(np.sort(pos)[:KEEP], M - 1)
        out[b] = order[pos]
    return out


def _nms_select_host(prob, w_fix):
    """Bit-exact replica of reference score+NMS, on jax CPU. Returns (NB, KEEP) int."""
    try:
        import jax
    except ImportError:
        return _nms_select_numpy(prob, w_fix)
    import jax.numpy as jnp
    from jax import lax

    cpu = jax.devices("cpu")[0]
    with jax.default_device(cpu):
        probj = jnp.asarray(prob)
        entropy = -jnp.sum(probj * jnp.log2(probj + 1e-10), axis=1)
        score = lax.conv(entropy[:, None], jnp.asarray(w_fix), (WIN // 4, WIN // 4), "VALID")
        score = (score / float((WIN // 2) ** 2)).reshape(NB, -1)
        iou = jnp.asarray(IOU_NP)

        def one(sc):
            order = jnp.argsort(-sc)
            iou_s = iou[order][:, order]

            def body(i, supp):
                return supp | ((~supp[i]) & (iou_s[i] > IOU_T) & (jnp.arange(M) > i))

            supp = lax.fori_loop(0, M, body, jnp.zeros((M,), dtype=bool))
            pos = jnp.where(~supp, jnp.arange(M), M)
            pos = jnp.minimum(jnp.sort(pos)[:KEEP], M - 1)
            return order[pos]

        idx = jax.vmap(one)(score)
        return np.asarray(jax.device_get(idx))


def _build_program(repeat=1):
    """Build+compile the shared SPMD Bass program (one batch per core).

    repeat>1 wraps the whole computation in an on-device loop (timing use
    only: amortizes host dispatch overhead; out is re-initialized from x at
    the top of every iteration so the final result is unchanged)."""
    nc = bacc.Bacc(
        "TRN2", target_bir_lowering=False, debug=False, num_devices=NB,
    )

    x = nc.dram_tensor("x", [NTOK, DIM], f32, kind="ExternalInput")
    wqkvT = nc.dram_tensor("wqkvT", [128, 4 * 3 * INNER], f32, kind="ExternalInput")
    woutT = nc.dram_tensor("woutT", [128, 4 * DIM], f32, kind="ExternalInput")
    tkT = nc.dram_tensor("tkT", [128, 512], f32, kind="ExternalInput")
    gsidx = nc.dram_tensor("gsidx", [128, 2 * KEEP], i32, kind="ExternalInput")
    invp = nc.dram_tensor("invp", [128, 2 * KEEP], f32, kind="ExternalInput")
    out = nc.dram_tensor("out", [NTOK, DIM], f32, kind="ExternalOutput")
    outw = nc.dram_tensor("outw", [KEEP * 256, DIM], f32) if _ABLATE == "nodma" else None

    EXP = mybir.ActivationFunctionType.Exp
    VST = 66  # per-head stride in the v tile: 64 dims + ones col + pad
    f32r = mybir.dt.float32r

    with tile.TileContext(nc) as tc:
        with (
            tc.tile_pool(name="cst", bufs=1) as cst,
            tc.tile_pool(name="sb", bufs=2) as sb,
            tc.tile_pool(name="ps", bufs=6, space="PSUM") as ps,
        ):
            stage = cst.tile([128, 4 * 3 * INNER], f32)
            nc.sync.dma_start(stage[:], wqkvT[:])
            wqkv_sb = cst.tile([128, 4 * 3 * INNER], f32r)
            nc.scalar.copy(wqkv_sb[:], stage[:])
            wout_sb = cst.tile([128, 4 * DIM], f32r)
            nc.sync.dma_start(stage[:, 0:4 * DIM], woutT[:])
            nc.scalar.copy(wout_sb[:], stage[:, 0:4 * DIM])
            tkT_sb = cst.tile([128, 512], f32r)
            nc.sync.dma_start(stage[:, 4 * DIM:4 * DIM + 512], tkT[:])
            nc.scalar.copy(tkT_sb[:], stage[:, 4 * DIM:4 * DIM + 512])
            idx_sb = cst.tile([128, 2 * KEEP], i32)
            nc.sync.dma_start(idx_sb[:], gsidx[:])
            inv_sb = cst.tile([128, 2 * KEEP], f32)
            nc.sync.dma_start(inv_sb[:], invp[:])
            ident = cst.tile([128, 128], f32)
            make_identity(nc, ident[:])
            ones_c = cst.tile([128, 16], f32)
            nc.vector.memset(ones_c[:], 1.0)

            def body():
                run_windows()

            def issue_gather(w):
                # token-major gather: partition p = token c*128+p of window w
                patch = sb.tile([128, 1024], f32r, tag="patch", bufs=4,
                                name=f"patch_{w}")
                for c in range(2):
                    nc.gpsimd.indirect_dma_start(
                        out=patch[:, c * 512:(c + 1) * 512],
                        out_offset=None,
                        in_=x[:, :],
                        in_offset=bass.IndirectOffsetOnAxis(
                            ap=idx_sb[:, 2 * w + c: 2 * w + c + 1], axis=0
                        ),
                    )
                return patch

            PF = 3  # gather prefetch depth (issued before the blocking scatters)

            def run_windows():
                # out starts as a copy of x; window results accumulate into it
                nc.sync.dma_start(out[:, :], x[:, :])
                pf = {}
                for w in range(min(PF, KEEP)):
                    pf[w] = issue_gather(w)
                for w in range(KEEP):
                    patch = pf.pop(w)
                    if w + PF < KEEP:
                        pf[w + PF] = issue_gather(w + PF)

                    # ---- ROI align: patT[dim, roi-token] = patch.T @ kron(B,B).T
                    patT = sb.tile([128, 1024], f32r, tag="patT", bufs=3)
                    for pair in range(2):  # dm pairs (0,1) and (2,3)
                        psA = ps.tile([128, 512], f32, tag="ps")
                        for half in range(2):
                            dm = pair * 2 + half
                            for c in range(2):
                                nc.tensor.matmul(
                                    psA[:, half * 256:(half + 1) * 256],
                                    lhsT=patch[:, c * 512 + dm * 128: c * 512 + (dm + 1) * 128],
                                    rhs=tkT_sb[:, c * 256:(c + 1) * 256],
                                    start=(c == 0),
                                    stop=(c == 1),
                                )
                        nc.scalar.copy(patT[:, pair * 512:(pair + 1) * 512], psA[:, :])

                    # ---- q,k in [inner, token] layout (blocks mi = inner 128-slices)
                    qkT = sb.tile([128, 2048], f32r, tag="qkT", bufs=3)
                    for pair in range(4):  # mi pairs
                        psB = ps.tile([128, 512], f32, tag="ps")
                        for half in range(2):
                            mi = pair * 2 + half
                            for dk in range(4):
                                nc.tensor.matmul(
                                    psB[:, half * 256:(half + 1) * 256],
                                    lhsT=wqkv_sb[:, dk * 1536 + mi * 128: dk * 1536 + (mi + 1) * 128],
                                    rhs=patT[:, dk * 256:(dk + 1) * 256],
                                    start=(dk == 0),
                                    stop=(dk == 3),
                                )
                        nc.scalar.copy(qkT[:, pair * 512:(pair + 1) * 512], psB[:, :])

                    # ---- v in [token, inner] layout with ones column per head
                    v_sb = sb.tile([128, 2 * 8 * VST], f32r, tag="v")
                    for mt in range(2):
                        psV = ps.tile([128, 512], f32, tag="ps")
                        for dk in range(4):
                            nc.tensor.matmul(
                                psV[:, :],
                                lhsT=patT[:, dk * 256 + mt * 128: dk * 256 + (mt + 1) * 128],
                                rhs=wqkv_sb[:, dk * 1536 + 1024: dk * 1536 + 1536],
                                start=(dk == 0),
                                stop=(dk == 3),
                            )
                        # strided copy into per-head 66-wide groups
                        base = v_sb[:, mt * 8 * VST: mt * 8 * VST + 8 * VST]
                        dst = bass.AP(
                            tensor=base.tensor,
                            offset=base.offset,
                            ap=[base.ap[0], [VST, 8], [1, 64]],
                        )
                        nc.vector.tensor_copy(dst, psV[:, :])
                        base1 = v_sb[:, mt * 8 * VST + 64: mt * 8 * VST + 64 + 1]
                        ones_dst = bass.AP(
                            tensor=base1.tensor,
                            offset=base1.offset,
                            ap=[base1.ap[0], [VST, 8], [1, 2]],
                        )
                        nc.vector.tensor_copy(ones_dst, ones_c[:, 0:16])

                    # ---- scores^T + exp:  E[ktok, qtok] = exp(0.125 * k.q)
                    e_all = sb.tile([128, 8 * 512], f32r, tag="e")
                    for h in range(8):
                        psS = ps.tile([128, 512], f32, tag="ps")
                        po = (h % 2) * 64
                        qb = h // 2
                        kb = 4 + h // 2
                        for kt in range(2):
                            nc.tensor.matmul(
                                psS[:, kt * 256:(kt + 1) * 256],
                                lhsT=qkT[po:po + 64, kb * 256 + kt * 128: kb * 256 + (kt + 1) * 128],
                                rhs=qkT[po:po + 64, qb * 256:(qb + 1) * 256],
                                start=True,
                                stop=True,
                            )
                        nc.scalar.activation(
                            e_all[:, h * 512:(h + 1) * 512], psS[:, :], EXP, scale=SCALE
                        )

                    # ---- attention @ [v | 1]:  out_q[qtok, 64+1] per head
                    out_q = [None, None]
                    for m in range(2):
                        out_q[m] = sb.tile([128, 512], f32, tag=f"outq{m}", name=f"outq{m}_{w}")
                        for hg in range(2):
                            psAV = ps.tile([128, 512], f32, tag="ps")
                            for h4 in range(4):
                                h = hg * 4 + h4
                                for kt in range(2):
                                    nc.tensor.matmul(
                                        psAV[:, h4 * VST: h4 * VST + 66],
                                        lhsT=e_all[:, h * 512 + kt * 256 + m * 128: h * 512 + kt * 256 + (m + 1) * 128],
                                        rhs=v_sb[:, kt * 8 * VST + h * VST: kt * 8 * VST + h * VST + 66],
                                        start=(kt == 0),
                                        stop=(kt == 1),
                                    )
                            rc = sb.tile([128, 4], f32, tag="rc")
                            for h4 in range(4):
                                nc.vector.reciprocal(
                                    rc[:, h4: h4 + 1], psAV[:, h4 * VST + 64: h4 * VST + 65]
                                )
                            for h4 in range(4):
                                nc.vector.tensor_scalar_mul(
                                    out_q[m][:, (hg * 4 + h4) * 64:(hg * 4 + h4 + 1) * 64],
                                    psAV[:, h4 * VST: h4 * VST + 64],
                                    rc[:, h4: h4 + 1],
                                )

                    # ---- transpose out_q -> outT [inner, token]
                    outT = sb.tile([128, 1024], f32r, tag="outT")
                    for m in range(2):
                        psT = ps.tile([128, 512], f32, tag="ps")
                        for ib in range(4):
                            nc.tensor.transpose(
                                psT[:, ib * 128:(ib + 1) * 128],
                                out_q[m][:, ib * 128:(ib + 1) * 128],
                                ident[:],
                            )
                        for ib in range(4):
                            nc.vector.tensor_copy(
                                outT[:, ib * 256 + m * 128: ib * 256 + (m + 1) * 128],
                                psT[:, ib * 128:(ib + 1) * 128],
                            )

                    # ---- output projection + 1/count, then scatter-accumulate
                    final = sb.tile([128, 1024], f32, tag="final")
                    for m in range(2):
                        psF = ps.tile([128, 512], f32, tag="ps")
                        for bk in range(4):
                            nc.tensor.matmul(
                                psF[:, :],
                                lhsT=outT[:, bk * 256 + m * 128: bk * 256 + (m + 1) * 128],
                                rhs=wout_sb[:, bk * 512:(bk + 1) * 512],
                                start=(bk == 0),
                                stop=(bk == 3),
                            )
                        nc.vector.tensor_scalar_mul(
                            final[:, m * 512:(m + 1) * 512],
                            psF[:, :],
                            inv_sb[:, 2 * w + m: 2 * w + m + 1],
                        )
                    for c in range(2):
                        if _ABLATE == "nodma":
                            nc.sync.dma_start(
                                outw[(2 * w + c) * 128:(2 * w + c + 1) * 128, :],
                                final[:, c * 512:(c + 1) * 512],
                            )
                        else:
                            nc.gpsimd.indirect_dma_start(
                                out=out[:, :],
                                out_offset=bass.IndirectOffsetOnAxis(
                                    ap=idx_sb[:, 2 * w + c: 2 * w + c + 1], axis=0
                                ),
                                in_=final[:, c * 512:(c + 1) * 512],
                                in_offset=None,
                                compute_op=mybir.AluOpType.add,
                            )

            if repeat == 1:
                body()
            else:
                with tc.For_i(0, repeat, 1):
                    body()
    nc.compile()
    return nc


_NC_CACHE = {}


def _get_program(repeat=1):
    if repeat not in _NC_CACHE:
        _NC_CACHE[repeat] = _build_program(repeat)
    return _NC_CACHE[repeat]


def _host_aux(idx):
    """Per-batch gather/scatter token tables + inverse-count patches."""
    p = np.arange(256)
    sy = SY_NP[idx]  # (KEEP,)
    sx = SX_NP[idx]
    # token (w, t) for t = 0..255: global = (sy + t//16)*128 + sx + t%16
    tok = (sy[:, None] + p[None] // WIN) * W + sx[:, None] + p[None] % WIN  # (KEEP,256)
    cnt = np.zeros(NTOK, np.float32)
    np.add.at(cnt, tok.ravel(), 1.0)
    inv = (np.float32(1.0) / (cnt + np.float32(1e-10))).astype(np.float32)
    gs = np.zeros((128, 2 * KEEP), np.int32)
    iv = np.zeros((128, 2 * KEEP), np.float32)
    for c in range(2):
        gs[:, c::2] = tok[:, c * 128:(c + 1) * 128].T
        iv[:, c::2] = inv[tok[:, c * 128:(c + 1) * 128]].T
    return gs, iv


def _prepare_in_maps(x, prob, W_fix, W_qkv, W_out, b_out):
    x = np.asarray(x, dtype=np.float32)
    prob = np.asarray(prob, dtype=np.float32)
    W_fix = np.asarray(W_fix, dtype=np.float32)
    W_qkv = np.asarray(W_qkv, dtype=np.float32)
    W_out = np.asarray(W_out, dtype=np.float32)

    idx = _nms_select_host(prob, W_fix)  # (NB, KEEP)

    # host-side SBUF layouts for the weights
    wqkvT = np.ascontiguousarray(
        W_qkv.T.reshape(4, 128, 3 * INNER).transpose(1, 0, 2).reshape(128, 4 * 3 * INNER)
    )
    woutT = np.ascontiguousarray(
        W_out.T.reshape(4, 128, DIM).transpose(1, 0, 2).reshape(128, 4 * DIM)
    )
    T = np.kron(BILIN_NP, BILIN_NP).astype(np.float32)  # (256 pq, 256 ij)
    TT = np.ascontiguousarray(T.T)  # (256 ij, 256 pq)
    tkT = np.ascontiguousarray(TT.reshape(2, 128, 256).transpose(1, 0, 2).reshape(128, 512))

    in_maps = []
    for b in range(NB):
        gs, iv = _host_aux(idx[b])
        in_maps.append(
            {
                "x": np.ascontiguousarray(x[b]),
                "wqkvT": wqkvT,
                "woutT": woutT,
                "tkT": tkT,
                "gsidx": gs,
                "invp": iv,
            }
        )
    return idx, in_maps


def _run(inputs, repeat=1, trace=False):
    idx, in_maps = _prepare_in_maps(**inputs)
    nc = _get_program(repeat)
    res = run_bass_kernel_spmd(
        nc, in_maps, core_ids=list(range(NB)), trace=trace
    )
    return idx, res


def kernel(x, prob, W_fix, W_qkv, W_out, b_out):
    b_out = np.asarray(b_out, dtype=np.float32)
    inputs = dict(x=x, prob=prob, W_fix=W_fix, W_qkv=W_qkv, W_out=W_out, b_out=b_out)
    idx, res = _run(inputs)
    out = np.stack([res.results[b]["out"] for b in range(NB)], 0)

    if np.any(b_out != 0.0):
        # bias contributes b_out once per covered token (cnt*inv == 1 exactly)
        for b in range(NB):
            gs, _ = _host_aux(idx[b])
            mask = np.zeros(NTOK, np.float32)
            mask[gs.ravel()] = 1.0
            out[b] += mask[:, None] * b_out[None, :]
    return out



# revision 30
# speedup vs baseline: 79.4414x; 79.4414x over previous
"""Sparse window attention (NMS-selected windows) for Trainium2, 8 cores.

Strategy:
- Host: replicate the (tiny) score/NMS control flow bit-exactly with jax-CPU,
  build per-batch gather/scatter token tables and 1/count patches.
- Device (1 batch per NeuronCore, SPMD over 8 cores): per selected window,
  indirect-DMA gather the 16x16 patch (token-major), ROI-align as a dense
  kron(BILIN,BILIN) matmul, qkv projection, attention (scores^T layout,
  softmax normalizer via an appended ones-column in V), output projection,
  multiply by 1/count and indirect-DMA scatter-accumulate into out (which is
  pre-initialized with x).
"""

import os
import sys

sys.path.insert(0, "/opt/trn_rl_repo")

_ABLATE = os.environ.get("KERNEL_ABLATE", "")

import numpy as np

import concourse.bass as bass
import concourse.bacc as bacc
import concourse.mybir as mybir
import concourse.tile as tile
from concourse.bass_utils import run_bass_kernel_spmd
from concourse.masks import make_identity

f32 = mybir.dt.float32
i32 = mybir.dt.int32

# ---- problem constants (must match reference.py) ----
H = W = 128
WIN = 16
DIM = 512
HEADS = 8
DIM_HEAD = 64
INNER = HEADS * DIM_HEAD
SCALE = DIM_HEAD ** -0.5
KEEP = 44
IOU_T = 0.2
NB = 8
NTOK = H * W  # 16384
M = 225

# static shifted windows
_s = np.arange(0, H - WIN + 1, WIN // 2)
_sx, _sy = np.meshgrid(_s, _s)
WINDOWS = np.stack(
    [_sx.ravel(), _sy.ravel(), _sx.ravel() + WIN - 1, _sy.ravel() + WIN - 1], 1
).astype(np.float32)
SX_NP = WINDOWS[:, 0].astype(np.int32)
SY_NP = WINDOWS[:, 1].astype(np.int32)

_x1, _y1, _x2, _y2 = WINDOWS[:, 0], WINDOWS[:, 1], WINDOWS[:, 2], WINDOWS[:, 3]
_area = (_x2 - _x1) * (_y2 - _y1)
_iw = np.clip(np.minimum(_x2[:, None], _x2[None]) - np.maximum(_x1[:, None], _x1[None]), 0, None)
_ih = np.clip(np.minimum(_y2[:, None], _y2[None]) - np.maximum(_y1[:, None], _y1[None]), 0, None)
_inter = _iw * _ih
IOU_NP = (_inter / (_area[:, None] + _area[None] - _inter)).astype(np.float32)

_bin = (WIN - 1.0) / WIN
_r = (np.arange(WIN) + 0.5) * _bin
_q = np.floor(_r).astype(int)
_f = (_r - _q).astype(np.float32)
_A = np.zeros((WIN, WIN), np.float32)
_A[np.arange(WIN), _q] = 1.0 - _f
_A[np.arange(WIN), np.minimum(_q + 1, WIN - 1)] += _f
BILIN_NP = _A  # (16,16)


def _nms_select_numpy(prob, w_fix):
    """Numpy fallback replica of the reference score+NMS."""
    entropy = -np.sum(prob * np.log2(prob + np.float32(1e-10)), axis=1)
    k = w_fix[0, 0]
    sc = np.zeros((NB, 15, 15), np.float32)
    for i in range(15):
        for j in range(15):
            sc[:, i, j] = np.sum(
                entropy[:, i * 4:i * 4 + 8, j * 4:j * 4 + 8] * k[None], axis=(1, 2)
            )
    score = (sc / np.float32(64.0)).reshape(NB, -1)
    out = np.zeros((NB, KEEP), np.int64)
    for b in range(NB):
        order = np.argsort(-score[b], kind="stable")
        iou_s = IOU_NP[order][:, order]
        supp = np.zeros(M, bool)
        for i in range(M):
            if not supp[i]:
                supp |= (iou_s[i] > IOU_T) & (np.arange(M) > i)
        pos = np.where(~supp, np.arange(M), M)
        pos = np.minimum(np.sort(pos)[:KEEP], M - 1)
        out[b] = order[pos]
    return out


def _nms_select_host(prob, w_fix):
    """Bit-exact replica of reference score+NMS, on jax CPU. Returns (NB, KEEP) int."""
    try:
        import jax
    except ImportError:
        return _nms_select_numpy(prob, w_fix)
    import jax.numpy as jnp
    from jax import lax

    cpu = jax.devices("cpu")[0]
    with jax.default_device(cpu):
        probj = jnp.asarray(prob)
        entropy = -jnp.sum(probj * jnp.log2(probj + 1e-10), axis=1)
        score = lax.conv(entropy[:, None], jnp.asarray(w_fix), (WIN // 4, WIN // 4), "VALID")
        score = (score / float((WIN // 2) ** 2)).reshape(NB, -1)
        iou = jnp.asarray(IOU_NP)

        def one(sc):
            order = jnp.argsort(-sc)
            iou_s = iou[order][:, order]

            def body(i, supp):
                return supp | ((~supp[i]) & (iou_s[i] > IOU_T) & (jnp.arange(M) > i))

            supp = lax.fori_loop(0, M, body, jnp.zeros((M,), dtype=bool))
            pos = jnp.where(~supp, jnp.arange(M), M)
            pos = jnp.minimum(jnp.sort(pos)[:KEEP], M - 1)
            return order[pos]

        idx = jax.vmap(one)(score)
        return np.asarray(jax.device_get(idx))


def _build_program(repeat=1):
    """Build+compile the shared SPMD Bass program (one batch per core).

    v2: bf16 matmul operands (FWL weight loads), row-paired score matmuls,
    batched DVE ops.  repeat>1 wraps the whole computation in an on-device
    loop (timing use only; out is re-initialized from x every iteration)."""
    nc = bacc.Bacc(
        "TRN2", target_bir_lowering=False, debug=False, num_devices=NB,
    )

    bf16 = mybir.dt.bfloat16
    f8 = mybir.dt.float8e4
    DR = mybir.MatmulPerfMode.DoubleRow
    x = nc.dram_tensor("x", [NTOK, DIM], f32, kind="ExternalInput")
    wqkvT = nc.dram_tensor("wqkvT", [128, 4 * 3 * INNER], f8, kind="ExternalInput")
    woutT = nc.dram_tensor("woutT", [128, 4 * DIM], f8, kind="ExternalInput")
    tkT = nc.dram_tensor("tkT", [128, 512], f32, kind="ExternalInput")
    gsidx = nc.dram_tensor("gsidx", [128, 2 * KEEP], i32, kind="ExternalInput")
    invp = nc.dram_tensor("invp", [128, 2 * KEEP], f32, kind="ExternalInput")
    out = nc.dram_tensor("out", [NTOK, DIM], f32, kind="ExternalOutput")
    outw = nc.dram_tensor("outw", [KEEP * 256, DIM], f32) if _ABLATE == "nodma" else None

    EXP = mybir.ActivationFunctionType.Exp
    VST = 66  # per-head stride in the v tile: 64 dims + ones col + pad
    f32r = mybir.dt.float32r

    with tile.TileContext(nc) as tc:
        with (
            tc.tile_pool(name="cst", bufs=1) as cst,
            tc.tile_pool(name="sb", bufs=2) as sb,
            tc.tile_pool(name="ps", bufs=8, space="PSUM") as ps,
        ):
            wqkv_sb = cst.tile([128, 4 * 3 * INNER], f8)
            nc.sync.dma_start(wqkv_sb[:], wqkvT[:])
            wout_sb = cst.tile([128, 4 * DIM], f8)
            nc.sync.dma_start(wout_sb[:], woutT[:])
            stage = cst.tile([128, 512], f32)
            tkT_sb = cst.tile([128, 512], f32r)
            nc.sync.dma_start(stage[:], tkT[:])
            nc.scalar.copy(tkT_sb[:], stage[:])
            idx_sb = cst.tile([128, 2 * KEEP], i32)
            nc.sync.dma_start(idx_sb[:], gsidx[:])
            inv_sb = cst.tile([128, 2 * KEEP], f32)
            nc.sync.dma_start(inv_sb[:], invp[:])
            ident = cst.tile([128, 128], bf16)
            make_identity(nc, ident[:])
            ones_c = cst.tile([128, 16], bf16)
            nc.vector.memset(ones_c[:], 1.0)

            def body():
                run_windows()

            def issue_gather(w):
                # token-major gather: partition p = token c*128+p of window w
                patch = sb.tile([128, 1024], f32r, tag="patch", bufs=7,
                                name=f"patch_{w}")
                for c in range(2):
                    nc.gpsimd.indirect_dma_start(
                        out=patch[:, c * 512:(c + 1) * 512],
                        out_offset=None,
                        in_=x[:, :],
                        in_offset=bass.IndirectOffsetOnAxis(
                            ap=idx_sb[:, 2 * w + c: 2 * w + c + 1], axis=0
                        ),
                    )
                return patch

            PF = 5  # gather prefetch depth

            INIT_CHUNKS = 16

            def run_windows():
                # Software pipeline: stages of consecutive windows are
                # interleaved so every PE consumer runs a full step after
                # its ACT/DVE producer (no PE stalls on evac latency).
                st = {}
                pf = {}
                for w in range(min(PF, KEEP)):
                    pf[w] = issue_gather(w)
                # out starts as a copy of x (chunked so the first gathers
                # aren't starved of DMA bandwidth at startup); window results
                # accumulate into it via the scatters, which Tile orders
                # after all init-chunk writes.
                # out starts as a copy of x, issued after the first gathers
                nc.sync.dma_start(out[:, :], x[:, :])
                for t in range(KEEP + 7):
                    if t < KEEP:
                        if t + PF < KEEP:
                            pf[t + PF] = issue_gather(t + PF)
                        st[t] = {"patch": pf.pop(t)}
                        stage_roi(t, st)
                    if 2 <= t and (t - 2) % 2 == 0 and t - 2 < KEEP:
                        stage_qkv_pair(t - 2, st)
                    if 0 <= t - 2 < KEEP:
                        stage_v(t - 2, st[t - 2])
                    if 0 <= t - 4 < KEEP:
                        stage_attn(t - 4, st[t - 4])
                    if 0 <= t - 5 < KEEP:
                        stage_transpose(t - 5, st[t - 5])
                    if 0 <= t - 6 < KEEP:
                        stage_final(t - 6, st[t - 6])
                        del st[t - 6]
                    if 0 <= t - 3 < KEEP:
                        stage_scores(t - 3, st[t - 3])

            def stage_roi(w, st):
                # ---- ROI align: patT2[dim, roi-token] = patch.T @ kron(B,B).T
                # pair layout: patT2[:, dm*512 + (w%2)*256 + pq]
                s = st[w]
                patch = s["patch"]
                if w % 2 == 0:
                    patT2 = sb.tile([128, 2048], f8, tag="patT", bufs=3)
                    s["patT"] = patT2
                else:
                    patT2 = st[w - 1]["patT"]
                    s["patT"] = patT2
                wi = w % 2
                for pair in range(2):  # dm pairs (0,1) and (2,3)
                    psA = ps.tile([128, 512], f32, tag="ps")
                    for half in range(2):
                        dm = pair * 2 + half
                        for c in range(2):
                            nc.tensor.matmul(
                                psA[:, half * 256:(half + 1) * 256],
                                lhsT=patch[:, c * 512 + dm * 128: c * 512 + (dm + 1) * 128],
                                rhs=tkT_sb[:, c * 256:(c + 1) * 256],
                                start=(c == 0),
                                stop=(c == 1),
                            )
                    nc.scalar.copy(
                        bass.AP(
                            tensor=patT2.tensor,
                            offset=patT2[:, pair * 1024 + wi * 256: pair * 1024 + wi * 256 + 1].offset,
                            ap=[patT2.ap[0], [512, 2], [1, 256]],
                        ),
                        psA[:, :],
                    )

            def stage_qkv_pair(w0, st):
                # ---- q,k for windows (w0, w0+1): [inner, token-pair] layout
                # qkT2[:, mi*512 + wi*256 + q]
                patT2 = st[w0]["patT"]
                w1 = min(w0 + 1, KEEP - 1)
                qkT2 = sb.tile([128, 4096], bf16, tag="qkT", bufs=3)
                st[w0]["qkT"] = qkT2
                if w1 > w0 and w1 in st:
                    st[w1]["qkT"] = qkT2
                for mi in range(8):
                    psB = ps.tile([128, 512], f32, tag="ps")
                    for dk2 in range(2):  # fp8 DoubleRow: contract 256 per mm
                        nc.tensor.matmul(
                            psB[:, :],
                            lhsT=bass.AP(
                                tensor=wqkv_sb.tensor,
                                offset=wqkv_sb[:, dk2 * 2 * 1536 + mi * 128: dk2 * 2 * 1536 + mi * 128 + 1].offset,
                                ap=[wqkv_sb.ap[0], [1536, 2], [1, 128]],
                            ),
                            rhs=bass.AP(
                                tensor=patT2.tensor,
                                offset=patT2[:, dk2 * 2 * 512: dk2 * 2 * 512 + 1].offset,
                                ap=[patT2.ap[0], [512, 2], [1, 512]],
                            ),
                            start=(dk2 == 0),
                            stop=(dk2 == 1),
                            perf_mode=DR,
                        )
                    if mi % 2 == 0:
                        nc.scalar.copy(qkT2[:, mi * 512:(mi + 1) * 512], psB[:, :])
                    else:
                        nc.vector.tensor_copy(qkT2[:, mi * 512:(mi + 1) * 512], psB[:, :])

            def stage_v(w, s):
                # ---- v in [token, inner] layout with ones column per head
                patT2 = s["patT"]
                wi = w % 2
                v_sb = sb.tile([128, 2 * 8 * VST], bf16, tag="v", bufs=4)
                s["v"] = v_sb
                for mt in range(2):
                    psV = ps.tile([128, 512], f32, tag="ps")
                    for dk2 in range(2):  # fp8 DoubleRow
                        nc.tensor.matmul(
                            psV[:, :],
                            lhsT=bass.AP(
                                tensor=patT2.tensor,
                                offset=patT2[:, dk2 * 2 * 512 + wi * 256 + mt * 128: dk2 * 2 * 512 + wi * 256 + mt * 128 + 1].offset,
                                ap=[patT2.ap[0], [512, 2], [1, 128]],
                            ),
                            rhs=bass.AP(
                                tensor=wqkv_sb.tensor,
                                offset=wqkv_sb[:, dk2 * 2 * 1536 + 1024: dk2 * 2 * 1536 + 1024 + 1].offset,
                                ap=[wqkv_sb.ap[0], [1536, 2], [1, 512]],
                            ),
                            start=(dk2 == 0),
                            stop=(dk2 == 1),
                            perf_mode=DR,
                        )
                    # strided copy into per-head 66-wide groups
                    base = v_sb[:, mt * 8 * VST: mt * 8 * VST + 8 * VST]
                    dst = bass.AP(
                        tensor=base.tensor,
                        offset=base.offset,
                        ap=[base.ap[0], [VST, 8], [1, 64]],
                    )
                    nc.vector.tensor_copy(dst, psV[:, :])
                    base1 = v_sb[:, mt * 8 * VST + 64: mt * 8 * VST + 64 + 1]
                    ones_dst = bass.AP(
                        tensor=base1.tensor,
                        offset=base1.offset,
                        ap=[base1.ap[0], [VST, 8], [1, 2]],
                    )
                    nc.vector.tensor_copy(ones_dst, ones_c[:, 0:16])

            def stage_scores(w, s):
                # ---- scores^T + exp:  E[ktok, qtok] = exp(0.125 * k.q)
                # heads (2j, 2j+1) sit at partition offsets 0/64 -> their
                # matmuls row-tile the PE array and run concurrently
                qkT = s["qkT"]
                wi = w % 2
                e_all = sb.tile([128, 8 * 512], bf16, tag="e", bufs=3)
                s["e"] = e_all
                for h in range(8):
                    psS = ps.tile([128, 512], f32, tag="ps")
                    po = (h % 2) * 64
                    qb = h // 2
                    kb = 4 + h // 2
                    for kt in range(2):
                        nc.tensor.matmul(
                            psS[:, kt * 256:(kt + 1) * 256],
                            lhsT=qkT[po:po + 64, kb * 512 + wi * 256 + kt * 128: kb * 512 + wi * 256 + (kt + 1) * 128],
                            rhs=qkT[po:po + 64, qb * 512 + wi * 256: qb * 512 + wi * 256 + 256],
                            start=True,
                            stop=True,
                        )
                    nc.scalar.activation(
                        e_all[:, h * 512:(h + 1) * 512], psS[:, :], EXP, scale=SCALE
                    )

            def stage_attn(w, s):
                # ---- attention @ [v | 1]:  out_q[qtok, 64+1] per head
                e_all = s["e"]
                v_sb = s["v"]
                out_q = [None, None]
                s["out_q"] = out_q
                for m in range(2):
                    out_q[m] = sb.tile([128, 512], bf16, tag=f"outq{m}", bufs=3,
                                       name=f"outq{m}_{w}")
                    for hg in range(2):
                        psAV = ps.tile([128, 512], f32, tag="ps")
                        for h4 in range(4):
                            h = hg * 4 + h4
                            for kt in range(2):
                                nc.tensor.matmul(
                                    psAV[:, h4 * VST: h4 * VST + 66],
                                    lhsT=e_all[:, h * 512 + kt * 256 + m * 128: h * 512 + kt * 256 + (m + 1) * 128],
                                    rhs=v_sb[:, kt * 8 * VST + h * VST: kt * 8 * VST + h * VST + 66],
                                    start=(kt == 0),
                                    stop=(kt == 1),
                                )
                        rc = sb.tile([128, 4], f32, tag="rc", bufs=6)
                        nc.vector.reciprocal(
                            rc[:, 0:4],
                            bass.AP(
                                tensor=psAV.tensor,
                                offset=psAV[:, 64:65].offset,
                                ap=[psAV.ap[0], [VST, 4]],
                            ),
                        )
                        nc.vector.tensor_tensor(
                            out=bass.AP(
                                tensor=out_q[m].tensor,
                                offset=out_q[m][:, hg * 256: hg * 256 + 1].offset,
                                ap=[out_q[m].ap[0], [64, 4], [1, 64]],
                            ),
                            in0=bass.AP(
                                tensor=psAV.tensor,
                                offset=psAV.offset,
                                ap=[psAV.ap[0], [VST, 4], [1, 64]],
                            ),
                            in1=bass.AP(
                                tensor=rc.tensor,
                                offset=rc.offset,
                                ap=[rc.ap[0], [1, 4], [0, 64]],
                            ),
                            op=mybir.AluOpType.mult,
                        )

            def stage_transpose(w, s):
                # ---- transpose out_q -> outT [inner, token]
                out_q = s["out_q"]
                outT = sb.tile([128, 1024], f8, tag="outT", bufs=3)
                s["outT"] = outT
                for m in range(2):
                    psT = ps.tile([128, 512], bf16, tag="ps")
                    for ib in range(4):
                        nc.tensor.transpose(
                            psT[:, ib * 128:(ib + 1) * 128],
                            out_q[m][:, ib * 128:(ib + 1) * 128],
                            ident[:],
                        )
                    nc.vector.tensor_copy(
                        bass.AP(
                            tensor=outT.tensor,
                            offset=outT[:, m * 128: m * 128 + 1].offset,
                            ap=[outT.ap[0], [256, 4], [1, 128]],
                        ),
                        psT[:, :],
                    )

            def stage_final(w, s):
                # ---- output projection + 1/count, then scatter-accumulate
                outT = s["outT"]
                final = sb.tile([128, 1024], f32, tag="final", bufs=3)
                for m in range(2):
                    psF = ps.tile([128, 512], f32, tag="ps")
                    for bk2 in range(2):  # fp8 DoubleRow
                        nc.tensor.matmul(
                            psF[:, :],
                            lhsT=bass.AP(
                                tensor=outT.tensor,
                                offset=outT[:, bk2 * 2 * 256 + m * 128: bk2 * 2 * 256 + m * 128 + 1].offset,
                                ap=[outT.ap[0], [256, 2], [1, 128]],
                            ),
                            rhs=bass.AP(
                                tensor=wout_sb.tensor,
                                offset=wout_sb[:, bk2 * 2 * 512: bk2 * 2 * 512 + 1].offset,
                                ap=[wout_sb.ap[0], [512, 2], [1, 512]],
                            ),
                            start=(bk2 == 0),
                            stop=(bk2 == 1),
                            perf_mode=DR,
                        )
                    nc.vector.tensor_scalar_mul(
                        final[:, m * 512:(m + 1) * 512],
                        psF[:, :],
                        inv_sb[:, 2 * w + m: 2 * w + m + 1],
                    )
                for c in range(2):
                    if _ABLATE == "nodma":
                        nc.sync.dma_start(
                            outw[(2 * w + c) * 128:(2 * w + c + 1) * 128, :],
                            final[:, c * 512:(c + 1) * 512],
                        )
                    else:
                        nc.gpsimd.indirect_dma_start(
                            out=out[:, :],
                            out_offset=bass.IndirectOffsetOnAxis(
                                ap=idx_sb[:, 2 * w + c: 2 * w + c + 1], axis=0
                            ),
                            in_=final[:, c * 512:(c + 1) * 512],
                            in_offset=None,
                            compute_op=mybir.AluOpType.add,
                        )

            if repeat == 1:
                body()
            else:
                with tc.For_i(0, repeat, 1):
                    body()
    nc.compile()
    return nc


_NC_CACHE = {}


def _get_program(repeat=1):
    if repeat not in _NC_CACHE:
        _NC_CACHE[repeat] = _build_program(repeat)
    return _NC_CACHE[repeat]


def _host_aux(idx):
    """Per-batch gather/scatter token tables + inverse-count patches."""
    p = np.arange(256)
    sy = SY_NP[idx]  # (KEEP,)
    sx = SX_NP[idx]
    # token (w, t) for t = 0..255: global = (sy + t//16)*128 + sx + t%16
    tok = (sy[:, None] + p[None] // WIN) * W + sx[:, None] + p[None] % WIN  # (KEEP,256)
    cnt = np.zeros(NTOK, np.float32)
    np.add.at(cnt, tok.ravel(), 1.0)
    inv = (np.float32(1.0) / (cnt + np.float32(1e-10))).astype(np.float32)
    gs = np.zeros((128, 2 * KEEP), np.int32)
    iv = np.zeros((128, 2 * KEEP), np.float32)
    for c in range(2):
        gs[:, c::2] = tok[:, c * 128:(c + 1) * 128].T
        iv[:, c::2] = inv[tok[:, c * 128:(c + 1) * 128]].T
    return gs, iv


def _prepare_in_maps(x, prob, W_fix, W_qkv, W_out, b_out):
    x = np.asarray(x, dtype=np.float32)
    prob = np.asarray(prob, dtype=np.float32)
    W_fix = np.asarray(W_fix, dtype=np.float32)
    W_qkv = np.asarray(W_qkv, dtype=np.float32)
    W_out = np.asarray(W_out, dtype=np.float32)

    idx = _nms_select_host(prob, W_fix)  # (NB, KEEP)

    # host-side SBUF layouts for the weights (fp8e4m3 for DoubleRow matmuls)
    f8_np = mybir.dt.np(mybir.dt.float8e4)
    wqkvT = np.ascontiguousarray(
        W_qkv.T.reshape(4, 128, 3 * INNER).transpose(1, 0, 2).reshape(128, 4 * 3 * INNER)
    ).astype(f8_np)
    woutT = np.ascontiguousarray(
        W_out.T.reshape(4, 128, DIM).transpose(1, 0, 2).reshape(128, 4 * DIM)
    ).astype(f8_np)
    T = np.kron(BILIN_NP, BILIN_NP).astype(np.float32)  # (256 pq, 256 ij)
    TT = np.ascontiguousarray(T.T)  # (256 ij, 256 pq)
    tkT = np.ascontiguousarray(TT.reshape(2, 128, 256).transpose(1, 0, 2).reshape(128, 512))

    in_maps = []
    for b in range(NB):
        gs, iv = _host_aux(idx[b])
        in_maps.append(
            {
                "x": np.ascontiguousarray(x[b]),
                "wqkvT": wqkvT,
                "woutT": woutT,
                "tkT": tkT,
                "gsidx": gs,
                "invp": iv,
            }
        )
    return idx, in_maps


def _run(inputs, repeat=1, trace=False):
    idx, in_maps = _prepare_in_maps(**inputs)
    nc = _get_program(repeat)
    res = run_bass_kernel_spmd(
        nc, in_maps, core_ids=list(range(NB)), trace=trace
    )
    return idx, res


def kernel(x, prob, W_fix, W_qkv, W_out, b_out):
    b_out = np.asarray(b_out, dtype=np.float32)
    inputs = dict(x=x, prob=prob, W_fix=W_fix, W_qkv=W_qkv, W_out=W_out, b_out=b_out)
    idx, res = _run(inputs)
    out = np.stack([res.results[b]["out"] for b in range(NB)], 0)

    if np.any(b_out != 0.0):
        # bias contributes b_out once per covered token (cnt*inv == 1 exactly)
        for b in range(NB):
            gs, _ = _host_aux(idx[b])
            mask = np.zeros(NTOK, np.float32)
            mask[gs.ravel()] = 1.0
            out[b] += mask[:, None] * b_out[None, :]
    return out



# revision 31
# speedup vs baseline: 81.4708x; 1.0255x over previous
"""Sparse window attention (NMS-selected windows) for Trainium2, 8 cores.

Strategy:
- Host: replicate the (tiny) score/NMS control flow bit-exactly with jax-CPU,
  build per-batch gather/scatter token tables and 1/count patches.
- Device (1 batch per NeuronCore, SPMD over 8 cores): per selected window,
  indirect-DMA gather the 16x16 patch (token-major), ROI-align as a dense
  kron(BILIN,BILIN) matmul, qkv projection, attention (scores^T layout,
  softmax normalizer via an appended ones-column in V), output projection,
  multiply by 1/count and indirect-DMA scatter-accumulate into out (which is
  pre-initialized with x).
"""

import os
import sys

sys.path.insert(0, "/opt/trn_rl_repo")

_ABLATE = os.environ.get("KERNEL_ABLATE", "")

import numpy as np

import concourse.bass as bass
import concourse.bacc as bacc
import concourse.mybir as mybir
import concourse.tile as tile
from concourse.bass_utils import run_bass_kernel_spmd
from concourse.masks import make_identity

f32 = mybir.dt.float32
i32 = mybir.dt.int32

# ---- problem constants (must match reference.py) ----
H = W = 128
WIN = 16
DIM = 512
HEADS = 8
DIM_HEAD = 64
INNER = HEADS * DIM_HEAD
SCALE = DIM_HEAD ** -0.5
KEEP = 44
IOU_T = 0.2
NB = 8
NTOK = H * W  # 16384
M = 225

# static shifted windows
_s = np.arange(0, H - WIN + 1, WIN // 2)
_sx, _sy = np.meshgrid(_s, _s)
WINDOWS = np.stack(
    [_sx.ravel(), _sy.ravel(), _sx.ravel() + WIN - 1, _sy.ravel() + WIN - 1], 1
).astype(np.float32)
SX_NP = WINDOWS[:, 0].astype(np.int32)
SY_NP = WINDOWS[:, 1].astype(np.int32)

_x1, _y1, _x2, _y2 = WINDOWS[:, 0], WINDOWS[:, 1], WINDOWS[:, 2], WINDOWS[:, 3]
_area = (_x2 - _x1) * (_y2 - _y1)
_iw = np.clip(np.minimum(_x2[:, None], _x2[None]) - np.maximum(_x1[:, None], _x1[None]), 0, None)
_ih = np.clip(np.minimum(_y2[:, None], _y2[None]) - np.maximum(_y1[:, None], _y1[None]), 0, None)
_inter = _iw * _ih
IOU_NP = (_inter / (_area[:, None] + _area[None] - _inter)).astype(np.float32)

_bin = (WIN - 1.0) / WIN
_r = (np.arange(WIN) + 0.5) * _bin
_q = np.floor(_r).astype(int)
_f = (_r - _q).astype(np.float32)
_A = np.zeros((WIN, WIN), np.float32)
_A[np.arange(WIN), _q] = 1.0 - _f
_A[np.arange(WIN), np.minimum(_q + 1, WIN - 1)] += _f
BILIN_NP = _A  # (16,16)


def _nms_select_numpy(prob, w_fix):
    """Numpy fallback replica of the reference score+NMS."""
    entropy = -np.sum(prob * np.log2(prob + np.float32(1e-10)), axis=1)
    k = w_fix[0, 0]
    sc = np.zeros((NB, 15, 15), np.float32)
    for i in range(15):
        for j in range(15):
            sc[:, i, j] = np.sum(
                entropy[:, i * 4:i * 4 + 8, j * 4:j * 4 + 8] * k[None], axis=(1, 2)
            )
    score = (sc / np.float32(64.0)).reshape(NB, -1)
    out = np.zeros((NB, KEEP), np.int64)
    for b in range(NB):
        order = np.argsort(-score[b], kind="stable")
        iou_s = IOU_NP[order][:, order]
        supp = np.zeros(M, bool)
        for i in range(M):
            if not supp[i]:
                supp |= (iou_s[i] > IOU_T) & (np.arange(M) > i)
        pos = np.where(~supp, np.arange(M), M)
        pos = np.minimum(np.sort(pos)[:KEEP], M - 1)
        out[b] = order[pos]
    return out


def _nms_select_host(prob, w_fix):
    """Bit-exact replica of reference score+NMS, on jax CPU. Returns (NB, KEEP) int."""
    try:
        import jax
    except ImportError:
        return _nms_select_numpy(prob, w_fix)
    import jax.numpy as jnp
    from jax import lax

    cpu = jax.devices("cpu")[0]
    with jax.default_device(cpu):
        probj = jnp.asarray(prob)
        entropy = -jnp.sum(probj * jnp.log2(probj + 1e-10), axis=1)
        score = lax.conv(entropy[:, None], jnp.asarray(w_fix), (WIN // 4, WIN // 4), "VALID")
        score = (score / float((WIN // 2) ** 2)).reshape(NB, -1)
        iou = jnp.asarray(IOU_NP)

        def one(sc):
            order = jnp.argsort(-sc)
            iou_s = iou[order][:, order]

            def body(i, supp):
                return supp | ((~supp[i]) & (iou_s[i] > IOU_T) & (jnp.arange(M) > i))

            supp = lax.fori_loop(0, M, body, jnp.zeros((M,), dtype=bool))
            pos = jnp.where(~supp, jnp.arange(M), M)
            pos = jnp.minimum(jnp.sort(pos)[:KEEP], M - 1)
            return order[pos]

        idx = jax.vmap(one)(score)
        return np.asarray(jax.device_get(idx))


def _build_program(repeat=1):
    """Build+compile the shared SPMD Bass program (one batch per core).

    v2: bf16 matmul operands (FWL weight loads), row-paired score matmuls,
    batched DVE ops.  repeat>1 wraps the whole computation in an on-device
    loop (timing use only; out is re-initialized from x every iteration)."""
    nc = bacc.Bacc(
        "TRN2", target_bir_lowering=False, debug=False, num_devices=NB,
    )

    bf16 = mybir.dt.bfloat16
    f8 = mybir.dt.float8e4
    DR = mybir.MatmulPerfMode.DoubleRow
    x = nc.dram_tensor("x", [NTOK, DIM], f32, kind="ExternalInput")
    wqkvT = nc.dram_tensor("wqkvT", [128, 4 * 3 * INNER], f8, kind="ExternalInput")
    woutT = nc.dram_tensor("woutT", [128, 4 * DIM], f8, kind="ExternalInput")
    tkT = nc.dram_tensor("tkT", [128, 512], f32, kind="ExternalInput")
    gsidx = nc.dram_tensor("gsidx", [128, 2 * KEEP], i32, kind="ExternalInput")
    invp = nc.dram_tensor("invp", [128, 2 * KEEP], f32, kind="ExternalInput")
    out = nc.dram_tensor("out", [NTOK, DIM], f32, kind="ExternalOutput")
    outw = nc.dram_tensor("outw", [KEEP * 256, DIM], f32) if _ABLATE == "nodma" else None

    EXP = mybir.ActivationFunctionType.Exp
    VST = 66  # per-head stride in the v tile: 64 dims + ones col + pad
    f32r = mybir.dt.float32r

    with tile.TileContext(nc) as tc:
        with (
            tc.tile_pool(name="cst", bufs=1) as cst,
            tc.tile_pool(name="sb", bufs=2) as sb,
            tc.tile_pool(name="ps", bufs=8, space="PSUM") as ps,
        ):
            wqkv_sb = cst.tile([128, 4 * 3 * INNER], f8)
            nc.sync.dma_start(wqkv_sb[:], wqkvT[:])
            wout_sb = cst.tile([128, 4 * DIM], f8)
            nc.sync.dma_start(wout_sb[:], woutT[:])
            stage = cst.tile([128, 512], f32)
            tkT_sb = cst.tile([128, 512], f32r)
            nc.sync.dma_start(stage[:], tkT[:])
            nc.scalar.copy(tkT_sb[:], stage[:])
            idx_sb = cst.tile([128, 2 * KEEP], i32)
            nc.sync.dma_start(idx_sb[:], gsidx[:])
            inv_sb = cst.tile([128, 2 * KEEP], f32)
            nc.sync.dma_start(inv_sb[:], invp[:])
            ident = cst.tile([128, 128], bf16)
            make_identity(nc, ident[:])
            ones_c = cst.tile([128, 16], f8)
            nc.vector.memset(ones_c[:], 1.0)

            def body():
                run_windows()

            def issue_gather(w):
                # token-major gather: partition p = token c*128+p of window w
                patch = sb.tile([128, 1024], f32r, tag="patch", bufs=7,
                                name=f"patch_{w}")
                for c in range(2):
                    nc.gpsimd.indirect_dma_start(
                        out=patch[:, c * 512:(c + 1) * 512],
                        out_offset=None,
                        in_=x[:, :],
                        in_offset=bass.IndirectOffsetOnAxis(
                            ap=idx_sb[:, 2 * w + c: 2 * w + c + 1], axis=0
                        ),
                    )
                return patch

            PF = 5  # gather prefetch depth

            INIT_CHUNKS = 16

            def run_windows():
                # Software pipeline: stages of consecutive windows are
                # interleaved so every PE consumer runs a full step after
                # its ACT/DVE producer (no PE stalls on evac latency).
                st = {}
                pf = {}
                for w in range(min(PF, KEEP)):
                    pf[w] = issue_gather(w)
                # out starts as a copy of x (chunked so the first gathers
                # aren't starved of DMA bandwidth at startup); window results
                # accumulate into it via the scatters, which Tile orders
                # after all init-chunk writes.
                # out starts as a copy of x, issued after the first gathers
                nc.sync.dma_start(out[:, :], x[:, :])
                for t in range(KEEP + 7):
                    if t < KEEP:
                        if t + PF < KEEP:
                            pf[t + PF] = issue_gather(t + PF)
                        st[t] = {"patch": pf.pop(t)}
                        stage_roi(t, st)
                    if 2 <= t and (t - 2) % 2 == 0 and t - 2 < KEEP:
                        stage_qkv_pair(t - 2, st)
                    if 0 <= t - 2 < KEEP:
                        stage_v(t - 2, st[t - 2])
                    if 0 <= t - 4 < KEEP:
                        stage_attn(t - 4, st[t - 4])
                    if 0 <= t - 5 < KEEP:
                        stage_transpose(t - 5, st[t - 5])
                    if 0 <= t - 6 < KEEP:
                        stage_final(t - 6, st[t - 6])
                        del st[t - 6]
                    if 0 <= t - 3 < KEEP:
                        stage_scores(t - 3, st[t - 3])

            def stage_roi(w, st):
                # ---- ROI align: patT2[dim, roi-token] = patch.T @ kron(B,B).T
                # pair layout: patT2[:, dm*512 + (w%2)*256 + pq]
                s = st[w]
                patch = s["patch"]
                if w % 2 == 0:
                    patT2 = sb.tile([128, 2048], f8, tag="patT", bufs=3)
                    s["patT"] = patT2
                else:
                    patT2 = st[w - 1]["patT"]
                    s["patT"] = patT2
                wi = w % 2
                for pair in range(2):  # dm pairs (0,1) and (2,3)
                    psA = ps.tile([128, 512], f32, tag="ps")
                    for half in range(2):
                        dm = pair * 2 + half
                        for c in range(2):
                            nc.tensor.matmul(
                                psA[:, half * 256:(half + 1) * 256],
                                lhsT=patch[:, c * 512 + dm * 128: c * 512 + (dm + 1) * 128],
                                rhs=tkT_sb[:, c * 256:(c + 1) * 256],
                                start=(c == 0),
                                stop=(c == 1),
                            )
                    nc.scalar.copy(
                        bass.AP(
                            tensor=patT2.tensor,
                            offset=patT2[:, pair * 1024 + wi * 256: pair * 1024 + wi * 256 + 1].offset,
                            ap=[patT2.ap[0], [512, 2], [1, 256]],
                        ),
                        psA[:, :],
                    )

            def stage_qkv_pair(w0, st):
                # ---- q,k for windows (w0, w0+1): [inner, token-pair] layout
                # qkT2[:, mi*512 + wi*256 + q]
                patT2 = st[w0]["patT"]
                w1 = min(w0 + 1, KEEP - 1)
                qkT2 = sb.tile([128, 4096], bf16, tag="qkT", bufs=3)
                st[w0]["qkT"] = qkT2
                if w1 > w0 and w1 in st:
                    st[w1]["qkT"] = qkT2
                for mi in range(8):
                    psB = ps.tile([128, 512], f32, tag="ps")
                    for dk2 in range(2):  # fp8 DoubleRow: contract 256 per mm
                        nc.tensor.matmul(
                            psB[:, :],
                            lhsT=bass.AP(
                                tensor=wqkv_sb.tensor,
                                offset=wqkv_sb[:, dk2 * 2 * 1536 + mi * 128: dk2 * 2 * 1536 + mi * 128 + 1].offset,
                                ap=[wqkv_sb.ap[0], [1536, 2], [1, 128]],
                            ),
                            rhs=bass.AP(
                                tensor=patT2.tensor,
                                offset=patT2[:, dk2 * 2 * 512: dk2 * 2 * 512 + 1].offset,
                                ap=[patT2.ap[0], [512, 2], [1, 512]],
                            ),
                            start=(dk2 == 0),
                            stop=(dk2 == 1),
                            perf_mode=DR,
                        )
                    if mi % 2 == 0:
                        nc.scalar.copy(qkT2[:, mi * 512:(mi + 1) * 512], psB[:, :])
                    else:
                        nc.vector.tensor_copy(qkT2[:, mi * 512:(mi + 1) * 512], psB[:, :])

            def stage_v(w, s):
                # ---- v in [token, inner] layout with ones column per head
                patT2 = s["patT"]
                wi = w % 2
                v_sb = sb.tile([128, 2 * 8 * VST], f8, tag="v", bufs=4)
                s["v"] = v_sb
                for mt in range(2):
                    psV = ps.tile([128, 512], f32, tag="ps")
                    for dk2 in range(2):  # fp8 DoubleRow
                        nc.tensor.matmul(
                            psV[:, :],
                            lhsT=bass.AP(
                                tensor=patT2.tensor,
                                offset=patT2[:, dk2 * 2 * 512 + wi * 256 + mt * 128: dk2 * 2 * 512 + wi * 256 + mt * 128 + 1].offset,
                                ap=[patT2.ap[0], [512, 2], [1, 128]],
                            ),
                            rhs=bass.AP(
                                tensor=wqkv_sb.tensor,
                                offset=wqkv_sb[:, dk2 * 2 * 1536 + 1024: dk2 * 2 * 1536 + 1024 + 1].offset,
                                ap=[wqkv_sb.ap[0], [1536, 2], [1, 512]],
                            ),
                            start=(dk2 == 0),
                            stop=(dk2 == 1),
                            perf_mode=DR,
                        )
                    # strided copy into per-head 66-wide groups
                    base = v_sb[:, mt * 8 * VST: mt * 8 * VST + 8 * VST]
                    dst = bass.AP(
                        tensor=base.tensor,
                        offset=base.offset,
                        ap=[base.ap[0], [VST, 8], [1, 64]],
                    )
                    nc.vector.tensor_copy(dst, psV[:, :])
                    base1 = v_sb[:, mt * 8 * VST + 64: mt * 8 * VST + 64 + 1]
                    ones_dst = bass.AP(
                        tensor=base1.tensor,
                        offset=base1.offset,
                        ap=[base1.ap[0], [VST, 8], [1, 2]],
                    )
                    nc.vector.tensor_copy(ones_dst, ones_c[:, 0:16])

            def stage_scores(w, s):
                # ---- scores^T + exp:  E[ktok, qtok] = exp(0.125 * k.q)
                # heads (2j, 2j+1) sit at partition offsets 0/64 -> their
                # matmuls row-tile the PE array and run concurrently
                qkT = s["qkT"]
                wi = w % 2
                e_all = sb.tile([128, 8 * 512], f8, tag="e", bufs=3)
                s["e"] = e_all
                for h in range(8):
                    psS = ps.tile([128, 512], f32, tag="ps")
                    po = (h % 2) * 64
                    qb = h // 2
                    kb = 4 + h // 2
                    for kt in range(2):
                        nc.tensor.matmul(
                            psS[:, kt * 256:(kt + 1) * 256],
                            lhsT=qkT[po:po + 64, kb * 512 + wi * 256 + kt * 128: kb * 512 + wi * 256 + (kt + 1) * 128],
                            rhs=qkT[po:po + 64, qb * 512 + wi * 256: qb * 512 + wi * 256 + 256],
                            start=True,
                            stop=True,
                        )
                    nc.scalar.activation(
                        e_all[:, h * 512:(h + 1) * 512], psS[:, :], EXP, scale=SCALE
                    )

            def stage_attn(w, s):
                # ---- attention @ [v | 1]:  out_q[qtok, 64+1] per head
                e_all = s["e"]
                v_sb = s["v"]
                out_q = [None, None]
                s["out_q"] = out_q
                for m in range(2):
                    out_q[m] = sb.tile([128, 512], bf16, tag=f"outq{m}", bufs=3,
                                       name=f"outq{m}_{w}")
                    for hg in range(2):
                        psAV = ps.tile([128, 512], f32, tag="ps")
                        for h4 in range(4):
                            h = hg * 4 + h4
                            for kt in range(2):
                                nc.tensor.matmul(
                                    psAV[:, h4 * VST: h4 * VST + 66],
                                    lhsT=e_all[:, h * 512 + kt * 256 + m * 128: h * 512 + kt * 256 + (m + 1) * 128],
                                    rhs=v_sb[:, kt * 8 * VST + h * VST: kt * 8 * VST + h * VST + 66],
                                    start=(kt == 0),
                                    stop=(kt == 1),
                                )
                        rc = sb.tile([128, 4], f32, tag="rc", bufs=6)
                        nc.vector.reciprocal(
                            rc[:, 0:4],
                            bass.AP(
                                tensor=psAV.tensor,
                                offset=psAV[:, 64:65].offset,
                                ap=[psAV.ap[0], [VST, 4]],
                            ),
                        )
                        nc.vector.tensor_tensor(
                            out=bass.AP(
                                tensor=out_q[m].tensor,
                                offset=out_q[m][:, hg * 256: hg * 256 + 1].offset,
                                ap=[out_q[m].ap[0], [64, 4], [1, 64]],
                            ),
                            in0=bass.AP(
                                tensor=psAV.tensor,
                                offset=psAV.offset,
                                ap=[psAV.ap[0], [VST, 4], [1, 64]],
                            ),
                            in1=bass.AP(
                                tensor=rc.tensor,
                                offset=rc.offset,
                                ap=[rc.ap[0], [1, 4], [0, 64]],
                            ),
                            op=mybir.AluOpType.mult,
                        )

            def stage_transpose(w, s):
                # ---- transpose out_q -> outT [inner, token]
                out_q = s["out_q"]
                outT = sb.tile([128, 1024], f8, tag="outT", bufs=3)
                s["outT"] = outT
                for m in range(2):
                    psT = ps.tile([128, 512], bf16, tag="ps")
                    for ib in range(4):
                        nc.tensor.transpose(
                            psT[:, ib * 128:(ib + 1) * 128],
                            out_q[m][:, ib * 128:(ib + 1) * 128],
                            ident[:],
                        )
                    nc.vector.tensor_copy(
                        bass.AP(
                            tensor=outT.tensor,
                            offset=outT[:, m * 128: m * 128 + 1].offset,
                            ap=[outT.ap[0], [256, 4], [1, 128]],
                        ),
                        psT[:, :],
                    )

            def stage_final(w, s):
                # ---- output projection + 1/count, then scatter-accumulate
                outT = s["outT"]
                final = sb.tile([128, 1024], f32, tag="final", bufs=3)
                for m in range(2):
                    psF = ps.tile([128, 512], f32, tag="ps")
                    for bk2 in range(2):  # fp8 DoubleRow
                        nc.tensor.matmul(
                            psF[:, :],
                            lhsT=bass.AP(
                                tensor=outT.tensor,
                                offset=outT[:, bk2 * 2 * 256 + m * 128: bk2 * 2 * 256 + m * 128 + 1].offset,
                                ap=[outT.ap[0], [256, 2], [1, 128]],
                            ),
                            rhs=bass.AP(
                                tensor=wout_sb.tensor,
                                offset=wout_sb[:, bk2 * 2 * 512: bk2 * 2 * 512 + 1].offset,
                                ap=[wout_sb.ap[0], [512, 2], [1, 512]],
                            ),
                            start=(bk2 == 0),
                            stop=(bk2 == 1),
                            perf_mode=DR,
                        )
                    nc.vector.tensor_scalar_mul(
                        final[:, m * 512:(m + 1) * 512],
                        psF[:, :],
                        inv_sb[:, 2 * w + m: 2 * w + m + 1],
                    )
                for c in range(2):
                    if _ABLATE == "nodma":
                        nc.sync.dma_start(
                            outw[(2 * w + c) * 128:(2 * w + c + 1) * 128, :],
                            final[:, c * 512:(c + 1) * 512],
                        )
                    else:
                        nc.gpsimd.indirect_dma_start(
                            out=out[:, :],
                            out_offset=bass.IndirectOffsetOnAxis(
                                ap=idx_sb[:, 2 * w + c: 2 * w + c + 1], axis=0
                            ),
                            in_=final[:, c * 512:(c + 1) * 512],
                            in_offset=None,
                            compute_op=mybir.AluOpType.add,
                        )

            if repeat == 1:
                body()
            else:
                with tc.For_i(0, repeat, 1):
                    body()
    nc.compile()
    return nc


_NC_CACHE = {}


def _get_program(repeat=1):
    if repeat not in _NC_CACHE:
        _NC_CACHE[repeat] = _build_program(repeat)
    return _NC_CACHE[repeat]


def _host_aux(idx):
    """Per-batch gather/scatter token tables + inverse-count patches."""
    p = np.arange(256)
    sy = SY_NP[idx]  # (KEEP,)
    sx = SX_NP[idx]
    # token (w, t) for t = 0..255: global = (sy + t//16)*128 + sx + t%16
    tok = (sy[:, None] + p[None] // WIN) * W + sx[:, None] + p[None] % WIN  # (KEEP,256)
    cnt = np.zeros(NTOK, np.float32)
    np.add.at(cnt, tok.ravel(), 1.0)
    inv = (np.float32(1.0) / (cnt + np.float32(1e-10))).astype(np.float32)
    gs = np.zeros((128, 2 * KEEP), np.int32)
    iv = np.zeros((128, 2 * KEEP), np.float32)
    for c in range(2):
        gs[:, c::2] = tok[:, c * 128:(c + 1) * 128].T
        iv[:, c::2] = inv[tok[:, c * 128:(c + 1) * 128]].T
    return gs, iv


def _prepare_in_maps(x, prob, W_fix, W_qkv, W_out, b_out):
    x = np.asarray(x, dtype=np.float32)
    prob = np.asarray(prob, dtype=np.float32)
    W_fix = np.asarray(W_fix, dtype=np.float32)
    W_qkv = np.asarray(W_qkv, dtype=np.float32)
    W_out = np.asarray(W_out, dtype=np.float32)

    idx = _nms_select_host(prob, W_fix)  # (NB, KEEP)

    # host-side SBUF layouts for the weights (fp8e4m3 for DoubleRow matmuls)
    f8_np = mybir.dt.np(mybir.dt.float8e4)
    wqkvT = np.ascontiguousarray(
        W_qkv.T.reshape(4, 128, 3 * INNER).transpose(1, 0, 2).reshape(128, 4 * 3 * INNER)
    ).astype(f8_np)
    woutT = np.ascontiguousarray(
        W_out.T.reshape(4, 128, DIM).transpose(1, 0, 2).reshape(128, 4 * DIM)
    ).astype(f8_np)
    T = np.kron(BILIN_NP, BILIN_NP).astype(np.float32)  # (256 pq, 256 ij)
    TT = np.ascontiguousarray(T.T)  # (256 ij, 256 pq)
    tkT = np.ascontiguousarray(TT.reshape(2, 128, 256).transpose(1, 0, 2).reshape(128, 512))

    in_maps = []
    for b in range(NB):
        gs, iv = _host_aux(idx[b])
        in_maps.append(
            {
                "x": np.ascontiguousarray(x[b]),
                "wqkvT": wqkvT,
                "woutT": woutT,
                "tkT": tkT,
                "gsidx": gs,
                "invp": iv,
            }
        )
    return idx, in_maps


def _run(inputs, repeat=1, trace=False):
    idx, in_maps = _prepare_in_maps(**inputs)
    nc = _get_program(repeat)
    res = run_bass_kernel_spmd(
        nc, in_maps, core_ids=list(range(NB)), trace=trace
    )
    return idx, res


def kernel(x, prob, W_fix, W_qkv, W_out, b_out):
    b_out = np.asarray(b_out, dtype=np.float32)
    inputs = dict(x=x, prob=prob, W_fix=W_fix, W_qkv=W_qkv, W_out=W_out, b_out=b_out)
    idx, res = _run(inputs)
    out = np.stack([res.results[b]["out"] for b in range(NB)], 0)

    if np.any(b_out != 0.0):
        # bias contributes b_out once per covered token (cnt*inv == 1 exactly)
        for b in range(NB):
            gs, _ = _host_aux(idx[b])
            mask = np.zeros(NTOK, np.float32)
            mask[gs.ravel()] = 1.0
            out[b] += mask[:, None] * b_out[None, :]
    return out

